# revision 64
# baseline (speedup 1.0000x reference)
"""Bass kernel for nn_Decoder (ragged tree-node decoder head), v2.

Everything foldable is folded on the HOST:
  G    = gelu(emb @ W_feats + b_feats)           [4096, 256]  (gather table)
  W1'  = diag(ln_g) W1, W2' = diag(ln_g) W2
  cb1  = ln_b @ W1 + b1 (row), cb2 = ln_b @ W2 + b2 (as per-partition column)
Device pipeline per 512-token tile (tokens-on-partitions, token = j*128+p):
  one 1024-row dma_gather from [G ; mem] concat table -> g-half, m-half
  x   = g + m                     (DVE, batched)
  bn_stats(x)                     (DVE)  -> SG-batched mean/var/rstd finish
  xn1 = (x - mu1) * rstd1         (DVE tensor_scalar)
  z1  = xn1 @ W1' + cb1           (PE: 8 transposes + 8 mm + 4 bias mm)
  h1  = gelu(z1)                  (ACT, from PSUM)
  bn_stats(h1), xn2 = LN2(h1), transpose
  z2T = W2'^T @ xn2T              (PE feature-major, 4 mm, N=512)
  h2T = gelu(z2T + cb2_col)       (ACT, bias per-partition)
  logits = h2T^T @ W_out          (PE, 8 mm, N=64, token-major)
  softmax: per-supergroup single EXP (ACT), batched reduce+recip (DVE),
  per-token scale (GPSIMD), one store DMA per supergroup (bf16 out).
Supergroup batching keeps ACT table loads to ~5 per 16 tiles."""

import math
from contextlib import ExitStack

import numpy as np

import concourse.bass as bass
from concourse import bacc
import concourse.mybir as mybir
import concourse.tile as tile
from concourse.masks import make_identity

F32 = mybir.dt.float32
BF16 = mybir.dt.bfloat16
I16 = mybir.dt.int16
AF = mybir.ActivationFunctionType
ALU = mybir.AluOpType
AX = mybir.AxisListType

D = 256
V = 64
NKB = D // 128  # 2 feature blocks
TILE = 512
NSUB = TILE // 128  # 4


def build_nc(T, NTAB, SG=16, has_cb1=True):
    NT = T // TILE
    assert T % TILE == 0
    nc = bacc.Bacc()

    gtab_d = nc.dram_tensor("gtab", [NTAB, D], BF16, kind="ExternalInput")
    memrep_d = nc.dram_tensor("memrep", [T, D], BF16, kind="ExternalInput")
    idx_d = nc.dram_tensor("idx", [128, NT * 32], I16, kind="ExternalInput")
    w1p_d = nc.dram_tensor("w1p", [128, NKB, D], BF16, kind="ExternalInput")
    w2p_d = nc.dram_tensor("w2p", [128, NKB, D], BF16, kind="ExternalInput")
    cb1_d = nc.dram_tensor("cb1", [1, D], BF16, kind="ExternalInput")
    cb2c_d = nc.dram_tensor("cb2c", [128, NKB], F32, kind="ExternalInput")
    wout_d = nc.dram_tensor("wout", [128, NKB, V], BF16, kind="ExternalInput")
    out_d = nc.dram_tensor("out", [T, V], BF16, kind="ExternalOutput")

    n_sg = math.ceil(NT / SG)

    with tile.TileContext(nc) as tc, ExitStack() as ctx:
        singles = ctx.enter_context(tc.tile_pool(name="singles", bufs=1))
        gpool = ctx.enter_context(tc.tile_pool(name="gpool", bufs=4))
        xbig = ctx.enter_context(tc.tile_pool(name="xbig", bufs=2))
        hbig = ctx.enter_context(tc.tile_pool(name="hbig", bufs=1))
        sfbig = ctx.enter_context(tc.tile_pool(name="sfbig", bufs=2))
        stats = ctx.enter_context(tc.tile_pool(name="stats", bufs=2))
        work = ctx.enter_context(tc.tile_pool(name="work", bufs=4))
        tpsum = ctx.enter_context(tc.tile_pool(name="tpsum", bufs=2, space="PSUM"))
        zp1 = ctx.enter_context(tc.tile_pool(name="zp1", bufs=2, space="PSUM"))
        zp2 = ctx.enter_context(tc.tile_pool(name="zp2", bufs=2, space="PSUM"))
        lps = ctx.enter_context(tc.tile_pool(name="lps", bufs=2, space="PSUM"))

        # ------- constants / weights -------
        ident = singles.tile([128, 128], BF16)
        make_identity(nc, ident)
        ones1 = singles.tile([1, 128], BF16)
        nc.vector.memset(ones1, 1.0)
        eps_sb = singles.tile([128, 1], F32)
        nc.vector.memset(eps_sb, 1e-5)

        w1p = singles.tile([128, NKB, D], BF16)
        nc.sync.dma_start(out=w1p, in_=w1p_d[:, :, :])
        w2p = singles.tile([128, NKB, D], BF16)
        nc.sync.dma_start(out=w2p, in_=w2p_d[:, :, :])
        cb1 = singles.tile([1, D], BF16)
        nc.sync.dma_start(out=cb1, in_=cb1_d[:, :])
        cb2c = singles.tile([128, NKB], F32)
        nc.sync.dma_start(out=cb2c, in_=cb2c_d[:, :])
        wout = singles.tile([128, NKB, V], BF16)
        nc.sync.dma_start(out=wout, in_=wout_d[:, :, :])
        idx_sb = singles.tile([128, NT * 32], I16)
        nc.sync.dma_start(out=idx_sb, in_=idx_d[:, :])

        L = SG * NSUB

        def stats_finish(bn, nt, tag):
            """bn [128, SG, NSUB, 6] -> (mu, rstd) [128, SG*NSUB] f32 packed.

            bn groups are (cnt, mean, n*var) for even / odd element halves;
            combine: mu = (me+mo)/2 ; M2 = M2e+M2o+64*(me-mo)^2 ;
            var = M2/256 ; rstd = 1/sqrt(var+eps)."""
            ln = nt * NSUB
            sl = (slice(None), slice(0, ln))
            me = bn[:, 0:nt, :, 1:2]
            mo = bn[:, 0:nt, :, 4:5]
            m2e = bn[:, 0:nt, :, 2:3]
            m2o = bn[:, 0:nt, :, 5:6]
            mu = stats.tile([128, L], F32, tag=f"mu{tag}")
            msum = stats.tile([128, L], F32, tag=f"ms{tag}")
            nc.vector.tensor_tensor(out=msum[sl], in0=me, in1=mo, op=ALU.add)
            nc.vector.tensor_scalar_mul(out=mu[sl], in0=msum[sl], scalar1=0.5)
            dm = stats.tile([128, L], F32, tag=f"dm{tag}")
            nc.vector.tensor_tensor(out=dm[sl], in0=me, in1=mo, op=ALU.subtract)
            dsq = stats.tile([128, L], F32, tag=f"dq{tag}")
            nc.vector.tensor_tensor(out=dsq[sl], in0=dm[sl], in1=dm[sl], op=ALU.mult)
            m2s = stats.tile([128, L], F32, tag=f"m2{tag}")
            nc.vector.tensor_tensor(out=m2s[sl], in0=m2e, in1=m2o, op=ALU.add)
            m2t = stats.tile([128, L], F32, tag=f"mt{tag}")
            nc.vector.scalar_tensor_tensor(
                out=m2t[sl], in0=dsq[sl], scalar=64.0, in1=m2s[sl],
                op0=ALU.mult, op1=ALU.add,
            )
            sd = stats.tile([128, L], F32, tag=f"sd{tag}")
            nc.scalar.activation(
                out=sd[sl], in_=m2t[sl], func=AF.Sqrt, bias=eps_sb, scale=1.0 / D
            )
            rstd = stats.tile([128, L], F32, tag=f"rs{tag}")
            nc.vector.reciprocal(out=rstd[sl], in_=sd[sl])
            negmur = stats.tile([128, L], F32, tag=f"nm{tag}")
            nc.vector.scalar_tensor_tensor(
                out=negmur[sl], in0=mu[sl], scalar=-1.0, in1=rstd[sl],
                op0=ALU.mult, op1=ALU.mult,
            )
            return mu, rstd, negmur

        for sg in range(n_sg):
            t0 = sg * SG
            nt = min(SG, NT - t0)
            tiles = range(t0, t0 + nt)

            xbuf = xbig.tile([128, SG, NSUB, D], BF16, tag="x")
            h1buf = hbig.tile([128, SG, NSUB, D], BF16, tag="h1")
            logbuf = sfbig.tile([128, SG, NSUB, V], BF16, tag="log")
            etbuf = sfbig.tile([128, SG, NSUB, V], BF16, tag="et")
            bn1 = stats.tile([128, SG, NSUB, 6], F32, tag="bn1")
            bn2 = stats.tile([128, SG, NSUB, 6], F32, tag="bn2")
            den = stats.tile([128, L], F32, tag="den")
            rd = stats.tile([128, L], F32, tag="rd")

            # ---- phase A: paired gather + mem DMA + add + stats ----
            for pi in range(0, nt, 2):
                npair = min(2, nt - pi)
                t = t0 + pi
                g = gpool.tile([128, 2 * NSUB, D], BF16, tag="g")
                nc.gpsimd.dma_gather(
                    out_ap=g[:, 0 : npair * NSUB, :],
                    in_ap=gtab_d[:, :],
                    idxs_ap=idx_sb[:, t * 32 : (t + npair) * 32],
                    num_idxs=npair * TILE,
                    num_idxs_reg=npair * TILE,
                    elem_size=D,
                    queue_num=0,
                )
                for q in range(npair):
                    ti = pi + q
                    tq = t + q
                    xm = gpool.tile([128, NSUB, D], BF16, tag="xm")
                    nc.sync.dma_start(
                        out=xm,
                        in_=memrep_d[tq * TILE : (tq + 1) * TILE, :].rearrange(
                            "(j p) e -> p j e", p=128
                        ),
                    )
                    nc.vector.tensor_tensor(
                        out=xbuf[:, ti], in0=g[:, q * NSUB : (q + 1) * NSUB, :],
                        in1=xm, op=ALU.add,
                    )
                    for j in range(NSUB):
                        nc.vector.bn_stats(out=bn1[:, ti, j], in_=xbuf[:, ti, j])

            # ---- phase B: LN1 stats finish (batched) ----
            mu1, rs1, nm1 = stats_finish(bn1, nt, 1)

            # ---- phase C: layer 1 ----
            for ti, t in enumerate(tiles):
                xn1 = work.tile([128, NSUB, D], BF16, tag="xn1")
                for j in range(NSUB):
                    c = ti * NSUB + j
                    nc.vector.tensor_scalar(
                        out=xn1[:, j, :], in0=xbuf[:, ti, j, :],
                        scalar1=mu1[:, c : c + 1], scalar2=rs1[:, c : c + 1],
                        op0=ALU.subtract, op1=ALU.mult,
                    )
                tp = tpsum.tile([128, NKB, TILE], BF16, tag="tp")
                for k in range(NKB):
                    for j in range(NSUB):
                        nc.tensor.transpose(
                            tp[:, k, j * 128 : (j + 1) * 128],
                            xn1[:, j, k * 128 : (k + 1) * 128],
                            ident,
                        )
                xn1t = work.tile([128, NKB, TILE], BF16, tag="xn1t")
                nc.scalar.activation(out=xn1t, in_=tp, func=AF.Copy)
                for half in range(2):
                    z1 = zp1.tile([128, 2, D], F32, tag="z1")
                    for jj in range(2):
                        j = half * 2 + jj
                        for k in range(NKB):
                            nc.tensor.matmul(
                                z1[:, jj, :],
                                xn1t[:, k, j * 128 : (j + 1) * 128],
                                w1p[:, k, :],
                                start=(k == 0),
                                stop=(k == NKB - 1) and not has_cb1,
                            )
                        if has_cb1:
                            nc.tensor.matmul(
                                z1[:, jj, :], ones1, cb1, start=False, stop=True
                            )
                    nc.scalar.activation(
                        out=h1buf[:, ti, half * 2 : half * 2 + 2, :], in_=z1,
                        func=AF.Gelu,
                    )
                for j in range(NSUB):
                    nc.vector.bn_stats(out=bn2[:, ti, j], in_=h1buf[:, ti, j])

            # ---- phase D: LN2 stats finish ----
            mu2, rs2, nm2 = stats_finish(bn2, nt, 2)

            # ---- phase E: layer 2 (feature-major) + head ----
            for ti, t in enumerate(tiles):
                xn2 = work.tile([128, NSUB, D], BF16, tag="xn2")
                for j in range(NSUB):
                    c = ti * NSUB + j
                    nc.vector.tensor_scalar(
                        out=xn2[:, j, :], in0=h1buf[:, ti, j, :],
                        scalar1=mu2[:, c : c + 1], scalar2=rs2[:, c : c + 1],
                        op0=ALU.subtract, op1=ALU.mult,
                    )
                tp2 = tpsum.tile([128, NKB, TILE], BF16, tag="tp")
                for k in range(NKB):
                    for j in range(NSUB):
                        nc.tensor.transpose(
                            tp2[:, k, j * 128 : (j + 1) * 128],
                            xn2[:, j, k * 128 : (k + 1) * 128],
                            ident,
                        )
                xn2t = work.tile([128, NKB, TILE], BF16, tag="xn2t")
                nc.scalar.activation(out=xn2t, in_=tp2, func=AF.Copy)
                h2t = work.tile([128, NKB, TILE], BF16, tag="h2t")
                for m in range(NKB):
                    z2 = zp2.tile([128, TILE], F32, tag="z2")
                    for k in range(NKB):
                        nc.tensor.matmul(
                            z2,
                            w2p[:, k, m * 128 : (m + 1) * 128],
                            xn2t[:, k, :],
                            start=(k == 0),
                            stop=(k == NKB - 1),
                        )
                    nc.scalar.activation(
                        out=h2t[:, m, :], in_=z2, func=AF.Gelu,
                        bias=cb2c[:, m : m + 1],
                    )
                lp = lps.tile([128, NSUB, V], F32, tag="lp")
                for j in range(NSUB):
                    for m in range(NKB):
                        nc.tensor.matmul(
                            lp[:, j, :],
                            h2t[:, m, j * 128 : (j + 1) * 128],
                            wout[:, m, :],
                            start=(m == 0),
                            stop=(m == NKB - 1),
                        )
                nc.vector.tensor_copy(logbuf[:, ti], lp)

            # ---- phase F: softmax (SG-batched) + store ----
            nc.scalar.activation(
                out=etbuf[:, 0:nt], in_=logbuf[:, 0:nt], func=AF.Exp
            )
            nc.vector.tensor_reduce(
                out=den[:, 0 : nt * NSUB], in_=etbuf[:, 0:nt], axis=AX.X, op=ALU.add
            )
            nc.vector.reciprocal(out=rd[:, 0 : nt * NSUB], in_=den[:, 0 : nt * NSUB])
            for ti, t in enumerate(tiles):
                for j in range(NSUB):
                    c = ti * NSUB + j
                    nc.vector.tensor_scalar_mul(
                        out=etbuf[:, ti, j, :], in0=etbuf[:, ti, j, :],
                        scalar1=rd[:, c : c + 1],
                    )
            nc.sync.dma_start(
                out=out_d[t0 * TILE : (t0 + nt) * TILE, :].rearrange(
                    "(tt j p) v -> p tt j v", p=128, j=NSUB
                ),
                in_=etbuf[:, 0:nt],
            )
    return nc


def wrap_idx(flat_idx):
    """dma_gather idx layout: slot i -> (partition i%16, col i//16), tiled
    to all 8 q7 groups."""
    base = np.asarray(flat_idx, dtype=np.int16).reshape(-1, 16).T
    return np.tile(base, (8, 1)).copy()


def _gelu_exact(x):
    from scipy.special import erf

    return 0.5 * x * (1.0 + erf(x / np.sqrt(2.0)))


def host_prep(inputs, n_cores=8):
    import ml_dtypes

    memory = np.asarray(inputs["memory"], np.float32)
    feat_idx = np.asarray(inputs["feat_idx"])
    emb = np.asarray(inputs["emb"], np.float32)
    W_feats = np.asarray(inputs["W_feats"], np.float32)
    b_feats = np.asarray(inputs["b_feats"], np.float32)
    ln_g = np.asarray(inputs["ln_g"], np.float32)
    ln_b = np.asarray(inputs["ln_b"], np.float32)
    W1 = np.asarray(inputs["W1"], np.float32)
    b1 = np.asarray(inputs["b1"], np.float32)
    W2 = np.asarray(inputs["W2"], np.float32)
    b2 = np.asarray(inputs["b2"], np.float32)
    W_out = np.asarray(inputs["W_out"], np.float32)

    Bq, Sq, Nn = feat_idx.shape
    Dm = memory.shape[-1]
    assert Dm == D
    bs_all = Bq * Sq
    bs_c = bs_all // n_cores
    T = bs_c * Nn  # tokens per core (not multiple of 512 in general)
    NT = math.ceil(T / TILE)
    Tpad = NT * TILE

    G = _gelu_exact(emb @ W_feats + b_feats).astype(ml_dtypes.bfloat16)
    VE = G.shape[0]
    W1p = (ln_g[:, None] * W1).astype(ml_dtypes.bfloat16)
    W2p = (ln_g[:, None] * W2).astype(ml_dtypes.bfloat16)
    cb1 = (ln_b @ W1 + b1).reshape(1, D).astype(ml_dtypes.bfloat16)
    cb2 = (ln_b @ W2 + b2).astype(np.float32)
    cb2c = cb2.reshape(NKB, 128).T.copy()  # [128, NKB]
    w1p = np.ascontiguousarray(
        W1p.reshape(NKB, 128, D).transpose(1, 0, 2)
    )  # [128, k, e]
    w2p = np.ascontiguousarray(W2p.reshape(NKB, 128, D).transpose(1, 0, 2))
    wout = np.ascontiguousarray(
        W_out.astype(ml_dtypes.bfloat16).reshape(NKB, 128, V).transpose(1, 0, 2)
    )

    mem_flat = memory.reshape(bs_all, D)
    fi_flat = feat_idx.reshape(bs_all, Nn)

    # token i (within a core) -> (bs row i//N, node i%N); padded tokens point
    # at row 0 / feat 0 (harmless, sliced off on the host).
    tok = np.arange(Tpad)
    bs_of_tok = np.where(tok < T, tok // Nn, 0).astype(np.int64)
    node_of_tok = np.where(tok < T, tok % Nn, 0).astype(np.int64)

    has_cb1 = bool(np.any(np.asarray(cb1, np.float32) != 0.0))
    in_maps = []
    shared = dict(w1p=w1p, w2p=w2p, cb1=cb1, cb2c=cb2c, wout=wout, gtab=G)
    for c in range(n_cores):
        mem_c = mem_flat[c * bs_c : (c + 1) * bs_c].astype(ml_dtypes.bfloat16)
        memrep = mem_c[bs_of_tok]  # [Tpad, D] pure replication
        fi_c = fi_flat[c * bs_c : (c + 1) * bs_c]
        gidx = fi_c[bs_of_tok, node_of_tok].astype(np.int64)
        in_maps.append(dict(shared, memrep=memrep, idx=wrap_idx(gidx)))
    return in_maps, dict(
        T=Tpad, Treal=T, NTAB=VE, bs_c=bs_c, Nn=Nn,
        B=Bq, S=Sq, n_cores=n_cores, has_cb1=has_cb1,
    )


def run_full(inputs, trace=False, sg=8):
    from concourse.bass_utils import run_bass_kernel_spmd

    in_maps, meta = host_prep(inputs)
    nc = build_nc(T=meta["T"], NTAB=meta["NTAB"], SG=sg, has_cb1=meta["has_cb1"])
    nc.finalize()
    res = run_bass_kernel_spmd(
        nc, in_maps, list(range(meta["n_cores"])), trace=trace
    )
    outs = []
    for c in range(meta["n_cores"]):
        o = np.asarray(res.results[c]["out"], dtype=np.float32)[: meta["Treal"]]
        outs.append(o.reshape(meta["bs_c"], meta["Nn"], V))
    out = np.concatenate(outs, axis=0)
    return out.reshape(meta["B"], meta["S"], meta["Nn"], V), res


def kernel(**inputs):
    out, _ = run_full(inputs, trace=False)
    return out.astype(np.float32)
